# revision 15
# baseline (speedup 1.0000x reference)
"""MAB (multihead attention block) TRN2 kernel.

Sharding: 8 cores = batch (4) x query-half (2). Each core computes its
[1024, 256] output slice with zero cross-core communication (K/V
projections are recomputed by the 2 cores sharing a batch).

Layout strategy: everything transposed (features on partitions) so that
- projections contract d on partitions,
- scores come out as [k, q] (exp output directly usable as A@V rhs),
- softmax denominators via a ones-row appended to V (PE does the sum),
- LN stats via ones-vector matmuls + K=1 broadcast matmuls (PE),
- FFN contracts e on partitions directly.
All matmuls run in float32r (~1.3e-4 rel err, full PE rate).
"""

import numpy as np

import concourse.bass as bass
import concourse.mybir as mybir
import concourse.tile as tile
from concourse import bacc
from concourse.bass_utils import run_bass_kernel_spmd

F32 = mybir.dt.float32
F32R = mybir.dt.float32r
AF = mybir.ActivationFunctionType
ALU = mybir.AluOpType

B, NQ, NK, D = 4, 2048, 2048, 256
H, DH = 4, 64
S = NQ // 2          # queries per core
ET = D // 128        # feature tiles
QB = S // 512        # query blocks of 512
KT = NK // 128       # key tiles of 128
KB = NK // 512       # key blocks of 512
EPS = 1e-5
SCALE = 1.0 / np.sqrt(D)

_CACHE = {}


def _build(flags):
    (use_bq, use_bk, use_bv, use_bo, use_g0, use_g1) = flags
    nc = bacc.Bacc(None, target_bir_lowering=False)

    dQ = nc.dram_tensor("Qs", [S, D], F32, kind="ExternalInput")
    dK = nc.dram_tensor("Ks", [NK, D], F32, kind="ExternalInput")
    dW = {w: nc.dram_tensor(w, [D, D], F32, kind="ExternalInput")
          for w in ("Wq", "Wk", "Wv", "Wo")}
    dV = {v: nc.dram_tensor(v, [D], F32, kind="ExternalInput")
          for v in ("bq", "bk", "bv", "bo", "g0", "b0", "g1", "b1")}
    dO = nc.dram_tensor("Out", [S, D], F32, kind="ExternalOutput")

    with tile.TileContext(nc) as tc:
        with (
            tc.tile_pool(name="const", bufs=1) as cpool,
            tc.tile_pool(name="acts", bufs=1) as apool,
            tc.tile_pool(name="big", bufs=1) as bpool,
        ):
            # ---------------- constants / weights ----------------
            w_r = {}
            for w in ("Wq", "Wk", "Wv", "Wo"):
                w0 = cpool.tile([128, ET, D], F32)
                for dt in range(ET):
                    nc.sync.dma_start(w0[:, dt, :], dW[w].rearrange("e (dt d) -> dt d e", d=128)[dt])
                wr = cpool.tile([128, ET, D], F32R)
                nc.vector.tensor_copy(wr[:], w0[:])
                w_r[w] = wr
            vecs = {}
            for v in ("bq", "bk", "bv", "bo", "g0", "b0", "g1", "b1"):
                t = cpool.tile([128, ET], F32)
                nc.sync.dma_start(t[:], dV[v].rearrange("(et e) -> e et", e=128))
                vecs[v] = t
            ones1 = cpool.tile([1, 128], F32)
            nc.vector.memset(ones1[:], 1.0)
            ones1r = cpool.tile([1, 128], F32R)
            nc.vector.tensor_copy(ones1r[:], ones1[:])
            onesc = cpool.tile([128, 1], F32)
            nc.vector.memset(onesc[:], 1.0 / D)
            onescr = cpool.tile([128, 1], F32R)
            nc.vector.tensor_copy(onescr[:], onesc[:])
            onesf0 = cpool.tile([128, 128], F32)
            nc.vector.memset(onesf0[:], 1.0)
            onesF = cpool.tile([128, 128], F32R)
            nc.vector.tensor_copy(onesF[:], onesf0[:])
            onesFb = cpool.tile([128, 1], mybir.dt.bfloat16)
            nc.vector.tensor_copy(onesFb[:], onesf0[:, 0:1])
            epst = cpool.tile([1, 1], F32)
            nc.vector.memset(epst[:], EPS)

            # ---------------- activations: load + round ----------------
            QT = apool.tile([128, ET, S], F32R)
            KTr = apool.tile([128, ET, NK], F32R)
            with tc.tile_pool(name="stage", bufs=1) as stpool:
                qt0 = stpool.tile([128, ET, S], F32)
                for dt in range(ET):
                    nc.sync.dma_start(qt0[:, dt, :], dQ.rearrange("s (dt d) -> dt d s", d=128)[dt])
                nc.vector.tensor_copy(QT[:], qt0[:])
                kt0 = stpool.tile([128, ET, NK], F32)
                for dt in range(ET):
                    nc.sync.dma_start(kt0[:, dt, :], dK.rearrange("s (dt d) -> dt d s", d=128)[dt])
                nc.vector.tensor_copy(KTr[:], kt0[:])

            qT = bpool.tile([128, ET, S], F32R)       # projected q, transposed
            kT = bpool.tile([128, ET, NK], F32R)      # projected k, transposed
            v_sb = bpool.tile([128, KT, D], F32R)  # v natural [k, e]
            OT = bpool.tile([128, ET, S], F32R)       # attention out + residual
            O1 = bpool.tile([128, ET, S], F32R)       # LN0 out
            O2 = bpool.tile([128, ET, S], F32R)       # FFN+residual out
            O3 = bpool.tile([128, ET, S], F32)        # LN1 out (final)

            # ---------------- phase A: projections ----------------
            with tc.tile_pool(name="psA", bufs=4, space="PSUM") as psA:
                for et in range(ET):
                    for qb in range(QB):
                        ps = psA.tile([128, 512], F32)
                        for dt in range(ET):
                            nc.tensor.matmul(
                                ps[:], w_r["Wq"][:, dt, et * 128:(et + 1) * 128],
                                QT[:, dt, qb * 512:(qb + 1) * 512],
                                start=(dt == 0), stop=(dt == ET - 1))
                        dst = qT[:, et, qb * 512:(qb + 1) * 512]
                        if use_bq:
                            nc.vector.tensor_scalar_add(dst, ps[:], vecs["bq"][:, et:et + 1])
                        else:
                            nc.vector.tensor_copy(dst, ps[:])
                for et in range(ET):
                    for kb in range(KB):
                        ps = psA.tile([128, 512], F32)
                        for dt in range(ET):
                            nc.tensor.matmul(
                                ps[:], w_r["Wk"][:, dt, et * 128:(et + 1) * 128],
                                KTr[:, dt, kb * 512:(kb + 1) * 512],
                                start=(dt == 0), stop=(dt == ET - 1))
                        dst = kT[:, et, kb * 512:(kb + 1) * 512]
                        if use_bk:
                            nc.vector.tensor_scalar_add(dst, ps[:], vecs["bk"][:, et:et + 1])
                        else:
                            nc.vector.tensor_copy(dst, ps[:])
                for kt in range(KT):
                    ps = psA.tile([128, 512], F32)
                    for dt in range(ET):
                        nc.tensor.matmul(
                            ps[:, 0:256], KTr[:, dt, kt * 128:(kt + 1) * 128],
                            w_r["Wv"][:, dt, :],
                            start=(dt == 0), stop=(dt == ET - 1))
                    nc.vector.tensor_copy(v_sb[:, kt, :], ps[:, 0:256])

            # ---------------- phase B: attention ----------------
            with (
                tc.tile_pool(name="scps", bufs=1, space="PSUM") as scps,
                tc.tile_pool(name="accps", bufs=1, space="PSUM") as accps,
                tc.tile_pool(name="bcps", bufs=2, space="PSUM") as bcps,
                tc.tile_pool(name="ut", bufs=3) as utp,
                tc.tile_pool(name="sm", bufs=2) as smp,
            ):
                for hp in range(2):          # head pair = e-tile of kT/qT
                    for qb in range(QB):
                        qsl = slice(qb * 512, (qb + 1) * 512)
                        acc = [accps.tile([64, 512], F32, name=f"acc{_h}", tag=f"acc{_h}")
                               for _h in range(2)]
                        sms = [accps.tile([1, 512], F32, name=f"sms{_h}", tag=f"sms{_h}")
                               for _h in range(2)]
                        for kt in range(KT):
                            sc = scps.tile([128, 1024], F32)
                            for hh in range(2):
                                off = hh * 64
                                nc.tensor.matmul(
                                    sc[:, hh * 512:(hh + 1) * 512],
                                    kT[off:off + 64, hp, kt * 128:(kt + 1) * 128],
                                    qT[off:off + 64, hp, qsl],
                                    start=True, stop=True)
                            ut = utp.tile([128, 1024], F32R)
                            nc.scalar.activation(ut[:], sc[:], AF.Exp, scale=SCALE)
                            for hh in range(2):
                                h = hp * 2 + hh
                                nc.tensor.matmul(
                                    acc[hh][:],
                                    v_sb[:, kt, h * 64:(h + 1) * 64],
                                    ut[:, hh * 512:(hh + 1) * 512],
                                    start=(kt == 0), stop=(kt == KT - 1))
                                nc.tensor.matmul(
                                    sms[hh][:],
                                    onesF[:, 0:1],
                                    ut[:, hh * 512:(hh + 1) * 512],
                                    start=(kt == 0), stop=(kt == KT - 1))
                        for hh in range(2):
                            rec = smp.tile([1, 512], F32, name=f"rec{hh}", tag="rec")
                            nc.vector.reciprocal_approx_fast(out=rec[:], in_=sms[hh][:])
                            recr = smp.tile([1, 512], F32R, name=f"recr{hh}", tag="recr")
                            nc.vector.tensor_copy(recr[:], rec[:])
                            recB = bcps.tile([64, 512], F32, name=f"recB{hh}", tag="recB")
                            nc.tensor.matmul(recB[:], onesF[0:1, 0:64], recr[:],
                                             start=True, stop=True)
                            recS = smp.tile([64, 512], F32, name=f"recS{hh}", tag="recS")
                            nc.vector.tensor_copy(recS[:], recB[:])
                            tmp = smp.tile([64, 512], F32, name=f"tmp{hh}", tag="tmp")
                            nc.vector.tensor_mul(tmp[:], acc[hh][:], recS[:])
                            if hh == 0:
                                nc.vector.tensor_add(OT[0:64, hp, qsl], tmp[:],
                                                     qT[0:64, hp, qsl])
                            else:
                                tsh = smp.tile([128, 512], F32, name="tsh", tag="tsh")
                                nc.sync.dma_start(tsh[64:128, :], tmp[:])
                                nc.vector.tensor_add(OT[64:128, hp, qsl], tsh[64:128, :],
                                                     qT[64:128, hp, qsl])
                        if use_bv:
                            nc.vector.tensor_scalar_add(OT[:, hp, qsl], OT[:, hp, qsl],
                                                        vecs["bv"][:, hp:hp + 1])

            # ---------------- phase C: LN0 -> FFN -> LN1 ----------------
            def layernorm(x, y, gname, bname, use_g, out_f32):
                with (
                    tc.tile_pool(name="lnps", bufs=2, space="PSUM") as lnps,
                    tc.tile_pool(name="lnbc", bufs=2, space="PSUM") as lnbc,
                    tc.tile_pool(name="lnsm", bufs=2) as lnsm,
                    tc.tile_pool(name="lnsq", bufs=2) as lnsq,
                ):
                    for qb in range(QB):
                        qsl = slice(qb * 512, (qb + 1) * 512)
                        xsq = lnsq.tile([128, ET, 512], F32R)
                        for et in range(ET):
                            nc.vector.tensor_mul(xsq[:, et, :], x[:, et, qsl], x[:, et, qsl])
                        mus = lnps.tile([1, 512], F32)
                        sqs = lnps.tile([1, 512], F32)
                        for et in range(ET):
                            nc.tensor.matmul(mus[:], onescr[:], x[:, et, qsl],
                                             start=(et == 0), stop=(et == ET - 1))
                            nc.tensor.matmul(sqs[:], onescr[:], xsq[:, et, :],
                                             start=(et == 0), stop=(et == ET - 1))
                        mu = lnsm.tile([1, 512], F32)
                        nc.vector.tensor_copy(mu[:], mus[:])
                        musq = lnsm.tile([1, 512], F32)
                        nc.vector.tensor_mul(musq[:], mu[:], mu[:])
                        var = lnsm.tile([1, 512], F32)
                        nc.vector.tensor_sub(var[:], sqs[:], musq[:])
                        sd = lnsm.tile([1, 512], F32)
                        nc.scalar.activation(sd[:], var[:], AF.Sqrt, bias=epst[:])
                        rst = lnsm.tile([1, 512], F32)
                        nc.vector.reciprocal_approx_fast(out=rst[:], in_=sd[:])
                        mur = lnsm.tile([1, 512], F32R)
                        nc.vector.tensor_copy(mur[:], mu[:])
                        rstr = lnsm.tile([1, 512], F32R)
                        nc.vector.tensor_copy(rstr[:], rst[:])
                        muB = lnbc.tile([128, 512], F32)
                        nc.tensor.matmul(muB[:], ones1r[:], mur[:], start=True, stop=True)
                        rsB = lnbc.tile([128, 512], F32)
                        nc.tensor.matmul(rsB[:], ones1r[:], rstr[:], start=True, stop=True)
                        for et in range(ET):
                            cen = lnsm.tile([128, 512], F32)
                            nc.vector.tensor_sub(cen[:], x[:, et, qsl], muB[:])
                            dst = y[:, et, qsl]
                            nc.vector.tensor_mul(dst, cen[:], rsB[:])
                            if use_g:
                                nc.vector.tensor_scalar(
                                    dst, dst, vecs[gname][:, et:et + 1],
                                    vecs[bname][:, et:et + 1], ALU.mult, ALU.add)

            layernorm(OT, O1, "g0", "b0", use_g0, False)

            with (
                tc.tile_pool(name="ffps", bufs=2, space="PSUM") as ffps,
                tc.tile_pool(name="ffsm", bufs=2) as ffsm,
            ):
                for et in range(ET):
                    for qb in range(QB):
                        qsl = slice(qb * 512, (qb + 1) * 512)
                        ps = ffps.tile([128, 512], F32)
                        for dt in range(ET):
                            nc.tensor.matmul(
                                ps[:], w_r["Wo"][:, dt, et * 128:(et + 1) * 128],
                                O1[:, dt, qsl],
                                start=(dt == 0), stop=(dt == ET - 1))
                        ft = ffsm.tile([128, 512], F32)
                        nc.vector.tensor_scalar(
                            ft[:], ps[:], vecs["bo"][:, et:et + 1] if use_bo else 0.0,
                            0.0, ALU.add, ALU.max)
                        nc.vector.tensor_add(O2[:, et, qsl], O1[:, et, qsl], ft[:])

            layernorm(O2, O3, "g1", "b1", use_g1, True)

            for et in range(ET):
                nc.sync.dma_start(
                    dO.rearrange("s (et e) -> et e s", e=128)[et], O3[:, et, :])

    nc.compile()
    return nc


def kernel(Q, K, Wq, bq, Wk, bk, Wv, bv, Wo, bo, g0, b0, g1, b1):
    Q, K = np.asarray(Q), np.asarray(K)
    ws = {n: np.ascontiguousarray(np.asarray(v), dtype=np.float32)
          for n, v in (("Wq", Wq), ("Wk", Wk), ("Wv", Wv), ("Wo", Wo))}
    vs = {n: np.ascontiguousarray(np.asarray(v), dtype=np.float32)
          for n, v in (("bq", bq), ("bk", bk), ("bv", bv), ("bo", bo),
                       ("g0", g0), ("b0", b0), ("g1", g1), ("b1", b1))}
    flags = (bool(np.any(vs["bq"])), bool(np.any(vs["bk"])),
             bool(np.any(vs["bv"])), bool(np.any(vs["bo"])),
             bool(np.any(vs["g0"] != 1.0) or np.any(vs["b0"])),
             bool(np.any(vs["g1"] != 1.0) or np.any(vs["b1"])))
    if flags not in _CACHE:
        _CACHE[flags] = _build(flags)
    nc = _CACHE[flags]

    in_maps = []
    for b in range(B):
        for half in range(2):
            m = {"Qs": np.ascontiguousarray(Q[b, half * S:(half + 1) * S], dtype=np.float32),
                 "Ks": np.ascontiguousarray(K[b], dtype=np.float32)}
            m.update(ws)
            m.update(vs)
            in_maps.append(m)

    res = run_bass_kernel_spmd(nc, in_maps, list(range(8)))
    out = np.empty((B, NQ, D), dtype=np.float32)
    for i in range(8):
        b, half = divmod(i, 2)
        out[b, half * S:(half + 1) * S] = res.results[i]["Out"]
    return out
